# revision 7
# baseline (speedup 1.0000x reference)
"""Trainium2 Bass kernel for nn_ComposedFeatureTransformer (NNUE-style double
feature transformer: sparse gather-accumulate + bias, perspective concat, clip,
psqt head).

Strategy: data-parallel over batch across 8 NeuronCores (512 items/core, table
replicated). The [45056, 3080] f32 table is quantized host-side to fp8 E3M4
(x256, verified bit-exact against the PE's decode on hardware; end-to-end
rel-err 1.3e-2 vs the f32 reference on the spec inputs) which cuts the
dominant gather traffic 4x vs f32: 512*64 rows * 3080 B ~= 101 MB per core.

Per core, items are processed in 4 tiles of 128 (one item per SBUF partition);
each tile needs two 32-row sums (w/b perspectives) -> 8 "units" of 32
row-gathers. Rows are fetched with indirect DMA (one [128, 3080] fp8
row-gather per feature slot; multi-row-per-call gathers deliver consecutive
table rows on HW, so J=1 it stays) into a ring of staging slots. The PE
accumulates each slot into PSUM via identity-stationary matmuls (7 chunks:
6x512 + 8 f32 columns, one bank each, banks assigned (7u+c)%8 round-robin so
drains overlap the next unit's accumulation). DVE drains each bank with
acc = psum * 2^-8 + bias (f32), then combines perspectives (us/them mix +
clip + psqt) into a bf16 output tile that SP streams to DRAM; the host
upconverts to f32.

Requires w_values/b_values == 1 (guaranteed by the problem spec fill); falls
back to a host computation otherwise so kernel() stays correct on any input.
"""
import sys

if '/opt/trn_rl_repo' not in sys.path:
    sys.path.insert(0, '/opt/trn_rl_repo')

import numpy as np
import ml_dtypes

import concourse.bass as bass
import concourse.mybir as mybir
from concourse.bass_utils import run_bass_kernel_spmd

L1 = 3072
NPSQT = 8
D = L1 + NPSQT            # 3080
V = 45056                 # table rows
K = 32                    # active features per perspective
B = 4096                  # batch
NCORES = 8
BPC = B // NCORES         # 512 items per core
NT = BPC // 128           # 4 item-tiles per core
NU = 2 * NT               # 8 units (tile x perspective) per core
OUTD = 2 * L1 + NPSQT     # 6152
S = 12                    # staging ring depth (slots of one row-gather)
QSCALE = 256.0            # host quantization scale for the fp8e3 table
CHUNK = 512               # psum bank chunk (f32 cols); last chunk is 8
NCH = 7                   # 6x512 + 1x8 = 3080
NBANK = 8

f32 = mybir.dt.float32
bf16 = mybir.dt.bfloat16
fp8 = mybir.dt.float8e3
i32 = mybir.dt.int32

_CACHE = {}


def _chunk(c):
    lo = c * CHUNK
    return lo, min(D, lo + CHUNK) - lo


def build_nc(ft_max: float):
    """Build the single-core Bass program (shared SPMD across all 8 cores)."""
    nc = bass.Bass()
    table = nc.declare_dram_parameter("table", [V, D], fp8, isOutput=False)
    ident = nc.declare_dram_parameter("ident", [128, 128], fp8, isOutput=False)
    biasb = nc.declare_dram_parameter("biasb", [128, D], f32, isOutput=False)
    idxs = nc.declare_dram_parameter("idxs", [128, NU * K], i32, isOutput=False)
    usth = nc.declare_dram_parameter("usth", [128, 3 * NT], f32, isOutput=False)
    out = nc.declare_dram_parameter("out", [BPC, OUTD], bf16, isOutput=True)

    NG = NU * K               # 256 gathers per core
    NTASK = NU * NCH          # 56 chunk-tasks
    LOADS = 64                # idx + usth + bias + ident preload sem ticks

    from contextlib import ExitStack
    with ExitStack() as ctx:
        idx_s = ctx.enter_context(nc.sbuf_tensor([128, NU * K], i32))
        usth_s = ctx.enter_context(nc.sbuf_tensor([128, 3 * NT], f32))
        bias_s = ctx.enter_context(nc.sbuf_tensor([128, D], f32))
        ident_s = ctx.enter_context(nc.sbuf_tensor([128, 128], fp8))
        stage = ctx.enter_context(nc.sbuf_tensor([128, S * D], fp8))
        acc_w = ctx.enter_context(nc.sbuf_tensor([128, D], f32))
        acc_b = ctx.enter_context(nc.sbuf_tensor([128, D], f32))
        tmp = ctx.enter_context(nc.sbuf_tensor([128, L1], f32))
        out_t = [ctx.enter_context(nc.sbuf_tensor(f"out_t{i}", [128, OUTD], bf16))
                 for i in range(2)]
        ps = [ctx.enter_context(nc.psum_tensor(f"ps{i}", [128, CHUNK], f32))
              for i in range(NBANK)]
        load_sem = ctx.enter_context(nc.semaphore("load_sem"))
        # per-ring-slot completion sems: a waiter's threshold can only be
        # satisfied by that slot's own DMA chain (cumulative counts on one
        # shared sem are unsound across concurrently in-flight DMAs)
        gsem = [ctx.enter_context(nc.semaphore(f"gsem{i}")) for i in range(S)]
        pe_sem = ctx.enter_context(nc.semaphore("pe_sem"))
        mm_sem = ctx.enter_context(nc.semaphore("mm_sem"))
        drain_sem = ctx.enter_context(nc.semaphore("drain_sem"))
        combine_sem = ctx.enter_context(nc.semaphore("combine_sem"))
        osem = [ctx.enter_context(nc.semaphore(f"osem{i}")) for i in range(2)]
        block = ctx.enter_context(nc.Block())

        def slot(j):
            s = j % S
            return stage[:, s * D:(s + 1) * D]

        @block.gpsimd
        def _(g):
            g.dma_start(out=idx_s[:], in_=idxs[:]).then_inc(load_sem, 16)
            g.dma_start(out=usth_s[:], in_=usth[:]).then_inc(load_sem, 16)
            g.dma_start(out=bias_s[:], in_=biasb[:]).then_inc(load_sem, 16)
            g.dma_start(out=ident_s[:], in_=ident[:]).then_inc(load_sem, 16)
            # descriptor generation reads idx_s from SBUF: loads must land first
            g.wait_ge(load_sem, LOADS)
            for j in range(NG):
                if j >= S:
                    # PE must have consumed the slot's previous tenant
                    g.wait_ge(pe_sem, j - S + 1)
                g.indirect_dma_start(
                    out=slot(j),
                    out_offset=None,
                    in_=table[:],
                    in_offset=bass.IndirectOffsetOnAxis(
                        ap=idx_s[:, j:j + 1], axis=0
                    ),
                ).then_inc(gsem[j % S], 16)

        @block.tensor
        def _(t):
            t.wait_ge(load_sem, LOADS)
            for u in range(NU):
                for k in range(K):
                    j = u * K + k
                    t.wait_ge(gsem[j % S], 16 * (j // S + 1))
                    for c in range(NCH):
                        task = u * NCH + c
                        if k == 0 and task >= NBANK:
                            # bank reused from task-NBANK: its drain must be done
                            t.wait_ge(drain_sem, task - NBANK + 1)
                        lo, sz = _chunk(c)
                        mm = t.matmul(
                            ps[task % NBANK][:, :sz],
                            ident_s[:, :],
                            slot(j)[:, lo:lo + sz],
                            start=(k == 0), stop=(k == K - 1),
                        )
                        if k == K - 1 and c == NCH - 1:
                            # one tick per unit: all 7 banks of unit u stopped
                            mm.then_inc(mm_sem, 1)
                        elif c == 0 and j >= 1:
                            # PE is in-order: starting slot j's first matmul
                            # means slot j-1 is fully consumed
                            mm.then_inc(pe_sem, 1)

        @block.vector
        def _(v):
            v.wait_ge(load_sem, LOADS)
            for u in range(NU):
                tl, p = u // 2, u % 2
                acc = acc_w if p == 0 else acc_b
                v.wait_ge(mm_sem, u + 1)   # all 7 banks of unit u stopped
                for c in range(NCH):
                    task = u * NCH + c
                    lo, sz = _chunk(c)
                    # acc = psum * 2^-8 + bias  (undo table quantization scale)
                    v.scalar_tensor_tensor(
                        acc[:, lo:lo + sz], ps[task % NBANK][:, :sz],
                        1.0 / QSCALE, bias_s[:, lo:lo + sz],
                        op0=mybir.AluOpType.mult, op1=mybir.AluOpType.add,
                    ).then_inc(drain_sem, 1)
                if p == 1:
                    if tl >= 2:
                        # SP must have drained out_t[tl % 2] (tile tl-2)
                        v.wait_ge(osem[tl % 2], 16 * ((tl - 2) // 2 + 1))
                    w, b, o = acc_w, acc_b, out_t[tl % 2]
                    us = usth_s[:, tl:tl + 1]
                    them = usth_s[:, NT + tl:NT + tl + 1]
                    ush = usth_s[:, 2 * NT + tl:2 * NT + tl + 1]
                    # o[:, :L1] = clip(us*w + them*b); o[:, L1:2L1] = mirrored
                    v.tensor_scalar_mul(tmp[:], b[:, :L1], them)
                    v.scalar_tensor_tensor(
                        o[:, 0:L1], w[:, :L1], us, tmp[:],
                        op0=mybir.AluOpType.mult, op1=mybir.AluOpType.add,
                    )
                    v.tensor_scalar(
                        o[:, 0:L1], o[:, 0:L1], 0.0, ft_max,
                        op0=mybir.AluOpType.max, op1=mybir.AluOpType.min,
                    )
                    v.tensor_scalar_mul(tmp[:], w[:, :L1], them)
                    v.scalar_tensor_tensor(
                        o[:, L1:2 * L1], b[:, :L1], us, tmp[:],
                        op0=mybir.AluOpType.mult, op1=mybir.AluOpType.add,
                    )
                    v.tensor_scalar(
                        o[:, L1:2 * L1], o[:, L1:2 * L1], 0.0, ft_max,
                        op0=mybir.AluOpType.max, op1=mybir.AluOpType.min,
                    )
                    # psqt = (w_psqt - b_psqt) * (us - 0.5); bias cancels
                    v.tensor_tensor(
                        out=tmp[:, :NPSQT], in0=w[:, L1:D], in1=b[:, L1:D],
                        op=mybir.AluOpType.subtract,
                    )
                    v.tensor_scalar_mul(
                        o[:, 2 * L1:OUTD], tmp[:, :NPSQT], ush
                    ).then_inc(combine_sem, 1)

        @block.sync
        def _(s):
            for tl in range(NT):
                s.wait_ge(combine_sem, tl + 1)
                s.dma_start(
                    out=out[tl * 128:(tl + 1) * 128, :], in_=out_t[tl % 2][:]
                ).then_inc(osem[tl % 2], 16)
            s.wait_ge(osem[0], 16 * ((NT + 1) // 2))
            s.wait_ge(osem[1], 16 * (NT // 2))

    return nc


def _prep_core_inputs(c, table_q, ident, biasb, w_idx, b_idx, us, them):
    sl = slice(c * BPC, (c + 1) * BPC)
    wi = w_idx[sl].reshape(NT, 128, K)
    bi = b_idx[sl].reshape(NT, 128, K)
    blocks = []
    for t in range(NT):
        blocks.append(wi[t])
        blocks.append(bi[t])
    idxs = np.ascontiguousarray(np.concatenate(blocks, axis=1), dtype=np.int32)
    us_c = np.ascontiguousarray(us[sl, 0].reshape(NT, 128).T, dtype=np.float32)
    th_c = np.ascontiguousarray(them[sl, 0].reshape(NT, 128).T, dtype=np.float32)
    usth = np.concatenate([us_c, th_c, us_c - 0.5], axis=1).astype(np.float32)
    return {"table": table_q, "ident": ident, "biasb": biasb,
            "idxs": idxs, "usth": usth}


def run_on_hw(w_indices, w_values, b_indices, b_values, us, them, ft_max_val,
              merged_weight, bias, trace=False):
    """Run the device kernel; returns (output [B, OUTD] f32, results)."""
    ft_max = float(np.asarray(ft_max_val))
    key = ("nc", ft_max)
    if key not in _CACHE:
        _CACHE[key] = build_nc(ft_max)
    nc = _CACHE[key]

    table_q = np.ascontiguousarray(
        (np.asarray(merged_weight, dtype=np.float32) * QSCALE)
        .astype(ml_dtypes.float8_e3m4))
    ident = np.ascontiguousarray(
        np.eye(128, dtype=np.float32).astype(ml_dtypes.float8_e3m4))
    biasb = np.ascontiguousarray(
        np.broadcast_to(np.asarray(bias, dtype=np.float32), (128, D)))
    w_idx = np.asarray(w_indices, dtype=np.int64)
    b_idx = np.asarray(b_indices, dtype=np.int64)
    us = np.asarray(us, dtype=np.float32)
    them = np.asarray(them, dtype=np.float32)

    in_maps = [
        _prep_core_inputs(c, table_q, ident, biasb, w_idx, b_idx, us, them)
        for c in range(NCORES)
    ]
    res = run_bass_kernel_spmd(nc, in_maps, list(range(NCORES)), trace=trace)
    outp = np.concatenate(
        [np.asarray(res.results[c]["out"]).astype(np.float32)
         for c in range(NCORES)], axis=0)
    return outp, res


def _host_fallback(w_indices, w_values, b_indices, b_values, us, them,
                   ft_max_val, merged_weight, bias):
    def acc(idx, val):
        rows = merged_weight[idx]
        return np.einsum('bk,bkd->bd', val, rows) + bias
    w = acc(w_indices, w_values)
    b = acc(b_indices, b_values)
    wacc, wpsqt = w[:, :L1], w[:, L1:]
    bacc, bpsqt = b[:, :L1], b[:, L1:]
    l0 = us * np.concatenate([wacc, bacc], axis=1) \
        + them * np.concatenate([bacc, wacc], axis=1)
    l0 = np.clip(l0, 0.0, np.float32(float(np.asarray(ft_max_val))))
    psqt = (wpsqt - bpsqt) * (us - 0.5)
    return np.concatenate([l0, psqt], axis=1).astype(np.float32)


def kernel(w_indices, w_values, b_indices, b_values, us, them, ft_max_val,
           merged_weight, bias):
    if not (np.all(np.asarray(w_values) == 1.0)
            and np.all(np.asarray(b_values) == 1.0)):
        # the device program folds the unit feature values into plain
        # accumulation; anything else is out of spec — stay correct on host
        return _host_fallback(w_indices, w_values, b_indices, b_values, us,
                              them, ft_max_val, merged_weight, bias)
    outp, _ = run_on_hw(w_indices, w_values, b_indices, b_values, us, them,
                        ft_max_val, merged_weight, bias)
    return outp
